# revision 29
# baseline (speedup 1.0000x reference)
"""MultiHeadChannelAttention Bass kernel for 8 Trainium2 NeuronCores.

Problem (hardcoded shapes): x (2, 512, 64, 32) fp32; Wq/Wk/Wv/Wfc (512, 512);
biases (512,). Reference math per batch b, with X = x[b].reshape(2048, 512):
  Q = X Wq^T + bq ; K = X Wk^T + bk ; V = X Wv^T + bv   (heads of 64 dims)
  out = softmax(QK^T/8) V  (per head), concat heads, @ Wfc^T + bfc

Sharding: 8 cores = 2 batches x 4 token-blocks of 512 tokens. Each core
computes K/V for all 2048 tokens of its batch (4x redundant), Q/attention/fc
only for its 512-token block. No cross-core communication; the host only
slices inputs and concatenates outputs. Tokens are rotated per-core so the
core's own block sits at columns 0:512 of X^T — the Q projection then reads
the same xt tiles as K/V and no separate xq tensor is shipped.

Device layouts (all matmul-friendly, weights pre-transposed on host):
  XT  [512c, 2048t]  = X^T (rotated), one full-width tile per 128-chan chunk
  KT [512, 2048] = (Wk X^T + bk);  QT [512, 512]
  scoresT [j, i] per head via row-tiled K=64 matmul pairs (2 heads/PE pass,
  concurrent in the PE array via distinct row groups)
  exp on ScalarE from 2-bank PSUM; on ~1/4 of tiles (always the last js
  of each pair, so boundary attnVs never wait on ScalarE) a DVE
  Schraudolph fast-exp (bf16-bit-pattern trick, one TENSOR_SCALAR) stands in
  so the exp stream isn't ScalarE-paced; attnV with ones-column (M=65) so the
  softmax denominator falls out of the same matmul; fc consumes attnout^T
  directly. bv is folded into the fc bias on host (softmax rows sum to 1).
  Softmax reciprocal = exp(-ln(x)) on ScalarE, batched [2, 512] per pair,
  broadcast to both head rows with a single fp16 selector matmul (fp32
  matmuls cost two half-speed PE passes). Output leaves as bf16; the host
  casts back to fp32.
"""

import numpy as np
import ml_dtypes

N_CORES = 8
B, C, N_TOK, TB = 2, 512, 2048, 512
HEADS, DK = 8, 64
NCH = C // 128  # channel chunks (4)
NJT = N_TOK // 128  # key-token tiles (16)
NTT = TB // 128  # fc token tiles (4)
HT = N_TOK // 2  # xt column-half (1024)

# Schraudolph fast-exp constants for bf16 bit patterns, including the 0.125
# attention scale: i16 = round(score * (0.125*128/ln2) + (127*128 - 5.5))
FEXP_C1 = 0.125 * 128.0 / float(np.log(2.0))
FEXP_C2 = 127.0 * 128.0 - 5.25
# iterations (per head-pair) whose exp runs on DVE instead of ScalarE.
# j=13/15 are always DVE so the pair's last attnVs never wait on the
# ScalarE exp queue at the boundary funnel.
FEXP_J = {
    0: (13, 15),
    1: (1, 7, 13, 15),
    2: (1, 7, 13, 15),
    3: (1, 3, 5, 9, 11, 13, 15),
}

_CACHE = {}


def _install_tile_drain_patch():
    """The end-of-kernel Tile drain can carry several sem waits; this
    walrus build allows one wait per non-EVSEM instruction. Split the
    waits across a chain of drains."""
    import bass_rust
    from concourse import tile as _tile
    from concourse.vector_clock import ScopedClock

    if getattr(_tile.TileContext, "_drain_patch_installed", False):
        return

    def _patched(self, tick_clock, wait_clock):
        nc = self.nc
        drain_inst = nc.sync.drain()
        wait_clock.add_sem_waits(
            drain_inst.ins, ScopedClock({None: tick_clock.global_clock})
        )
        si = drain_inst.ins.sync_info
        if si is not None and len(si.on_wait) > 1:
            waits = list(si.on_wait)
            drain_inst.ins.sync_info = bass_rust.SyncInfo(
                on_wait=[waits[0]], on_update=list(si.on_update)
            )
            for w in waits[1:]:
                extra = nc.sync.drain()
                extra.ins.sync_info = bass_rust.SyncInfo(on_wait=[w], on_update=[])
        nc.all_engine_barrier()
        assert self.sems is not None
        popped = nc._tile_sem_poison_stack.pop()
        assert popped is self._sem_poison
        nc.clear_and_free_semaphores(list(self.sems.allocated().values()))
        nc.all_engine_barrier()

    _tile.TileContext._drain_and_barrier = _patched
    _tile.TileContext._drain_patch_installed = True


def _split_multi_waits(nc):
    """This walrus build accepts one sync wait per instruction (two on
    EVSEM). Tile can attach two; move extras onto preceding NOPs."""
    import concourse.mybir as mybir

    for f in nc.m.functions:
        for bb in f.blocks:
            out = []
            changed = False
            for ins in bb.instructions:
                si = ins.sync_info
                limit = 2 if isinstance(ins, mybir.InstEventSemaphore) else 1
                if si is not None and len(si.on_wait) > limit:
                    waits = list(si.on_wait)
                    keep = waits[-limit:]
                    for i, w in enumerate(waits[:-limit]):
                        nop = mybir.InstNoOp(
                            name=f"{ins.name}_w{i}",
                            engine=ins.engine,
                            sync_info=mybir.SyncInfo(on_wait=[w], on_update=[]),
                            bass_nofuse=True,
                        )
                        nc.register_instruction(nop, overwrite=True)
                        out.append(nop)
                    ins.sync_info = mybir.SyncInfo(
                        on_wait=keep, on_update=list(si.on_update)
                    )
                    changed = True
                out.append(ins)
            if changed:
                bb.instructions = out


def _build():
    import concourse.bass as bass
    import concourse.mybir as mybir
    import concourse.tile as tile
    from concourse.bass import ts

    dt = mybir.dt
    f32, bf16, i16, f16 = dt.float32, dt.bfloat16, dt.int16, dt.float16
    Exp = mybir.ActivationFunctionType.Exp
    Ln = mybir.ActivationFunctionType.Ln
    Mult, Add = mybir.AluOpType.mult, mybir.AluOpType.add

    nc = bass.Bass()
    # weights are host-interleaved to [128, NCH*cols] so each DMA moves
    # one big per-partition span (large DMA packets) while chunk c still
    # slices out as [:, c*cols : ...] with partition p = channel 128c+p
    xt_d = nc.dram_tensor("xt", [C, N_TOK], bf16, kind="ExternalInput")
    wqT_d = nc.dram_tensor("wqT", [128, NCH * C], bf16, kind="ExternalInput")
    wkT_d = nc.dram_tensor("wkT", [128, NCH * C], bf16, kind="ExternalInput")
    wvT_d = nc.dram_tensor("wvT", [128, NCH * C], bf16, kind="ExternalInput")
    wfT_d = nc.dram_tensor("wfT", [128, NCH * C], bf16, kind="ExternalInput")
    bias_d = nc.dram_tensor("bias", [128, 2 * NCH], f32, kind="ExternalInput")
    bfc_d = nc.dram_tensor("bfc", [1, C], bf16, kind="ExternalInput")
    out_d = nc.dram_tensor("out", [TB, C], bf16, kind="ExternalOutput")

    with tile.TileContext(nc) as tc:
        with (
            tc.tile_pool(name="wp", bufs=1) as wp,
            tc.tile_pool(name="data", bufs=1) as data,
            tc.tile_pool(name="ep", bufs=6) as ep,
            tc.tile_pool(name="np_", bufs=2) as npool,
            tc.tile_pool(name="scp", bufs=2, space=bass.MemorySpace.PSUM) as scp,
            tc.tile_pool(name="ap_", bufs=1, space=bass.MemorySpace.PSUM) as apool,
            tc.tile_pool(name="aux", bufs=2, space=bass.MemorySpace.PSUM) as aux,
        ):
            # ---- constants / weights (merged [128, NCH*cols] tiles).
            # wq/wk are d-major ((d, c) 128-col blocks) so the d=0 pieces
            # that gate the first scores ship as small early DMAs; wv/wf
            # stay c-major (consumed whole-chunk) ----
            wq_all = wp.tile([128, NCH * C], bf16, tag="wq", name="wq_all")
            wk_all = wp.tile([128, NCH * C], bf16, tag="wk", name="wk_all")
            wv_all = wp.tile([128, NCH * C], bf16, tag="wv", name="wv_all")
            wf_all = wp.tile([128, NCH * C], bf16, tag="wf", name="wf_all")

            def wqd(d, c):
                return wq_all[:, ts(d * NCH + c, 128)]

            def wkd(d, c):
                return wk_all[:, ts(d * NCH + c, 128)]

            wv = [wv_all[:, ts(c, C)] for c in range(NCH)]
            wf = [wf_all[:, ts(c, C)] for c in range(NCH)]
            bias_all = wp.tile([128, 2 * NCH], f32, tag="bias", name="bias_all")
            bqt = [bias_all[:, d : d + 1] for d in range(NCH)]
            bkt = [bias_all[:, NCH + d : NCH + d + 1] for d in range(NCH)]
            bfct = wp.tile([1, C], bf16, tag="bfct", name="bfct")
            ones_t = wp.tile([128, TB], bf16, tag="ones", name="ones_t")
            nc.gpsimd.memset(ones_t[:], 1.0)
            ones_f = wp.tile([128, 64], f32, tag="onesf", name="ones_f")
            nc.vector.memset(ones_f[:], 1.0)
            # selector for the denominator broadcast: one fp16 matmul maps
            # rcp row 0 -> out partitions 0-63 and row 32 -> 64-127
            sel = wp.tile([33, 128], f16, tag="sel", name="sel")
            nc.vector.memset(sel[:], 0.0)
            nc.vector.memset(sel[0:1, 0:64], 1.0)
            nc.vector.memset(sel[32:33, 64:128], 1.0)

            # ---- activations in: one full-width tile per channel chunk.
            # Full 4KB DRAM rows: DMA cost is ~27ns per partition-row
            # regardless of row bytes (up to 4KB), so half-width tiles
            # would double the load time for the same data ----
            xt_sb = [
                data.tile([128, N_TOK], bf16, tag=f"xt{c}", name=f"xt{c}")
                for c in range(NCH)
            ]

            # ---- input DMAs over the three issue paths (SP/ACT HWDGE +
            # gpsimd SWDGE), each ring ~27ns/partition-row. Ordered by
            # consumption deadline: the d=0 weight blocks + all xt gate the
            # first scores (~15.5us); chunk 1 is row-split across two rings
            # so the last xt piece lands ~14.5us; wv gates the first attnV
            # (~18us); wq/wk d1-3 by the pair-0 boundary; wf only at the fc
            # prefill (~100us) ----
            nc.sync.dma_start(out=bias_all[:], in_=bias_d[:])
            nc.scalar.dma_start(out=wq_all[:, 0 : NCH * 128], in_=wqT_d[:, 0 : NCH * 128])
            nc.scalar.dma_start(out=wk_all[:, 0 : NCH * 128], in_=wkT_d[:, 0 : NCH * 128])
            nc.sync.dma_start(out=xt_sb[0][:], in_=xt_d[ts(0, 128), :])
            nc.gpsimd.dma_start(out=xt_sb[2][:], in_=xt_d[ts(2, 128), :])
            nc.sync.dma_start(out=xt_sb[1][0:64, :], in_=xt_d[128:192, :])
            nc.gpsimd.dma_start(out=xt_sb[1][64:128, :], in_=xt_d[192:256, :])
            nc.scalar.dma_start(out=xt_sb[3][:], in_=xt_d[ts(3, 128), :])
            nc.scalar.dma_start(out=wv_all[:], in_=wvT_d[:])
            nc.gpsimd.dma_start(out=wf_all[:], in_=wfT_d[:])
            nc.scalar.dma_start(
                out=wq_all[:, NCH * 128 : NCH * C], in_=wqT_d[:, NCH * 128 : NCH * C]
            )
            nc.scalar.dma_start(
                out=wk_all[:, NCH * 128 : NCH * C], in_=wkT_d[:, NCH * 128 : NCH * C]
            )
            nc.sync.dma_start(out=bfct[:], in_=bfc_d[:])

            # trigger the natural_log_exp ACT table load during the DMA
            # window instead of right before the first real exp
            tbl = npool.tile([1, 64], f32, tag="tbl", bufs=1, name="tbl")
            nc.scalar.activation(out=tbl[:], in_=ones_f[0:1, :], func=Ln)

            # PE warmup: one dummy accumulation chain on the ones tile keeps
            # the HAM activity monitor busy through the input-load window so
            # the first real projections run at 2.4 GHz
            warm = aux.tile([128, TB], f32, tag="aux", name="warm")
            for r in range(10):
                nc.tensor.matmul(
                    warm[:], ones_t[0:1, 0:128], ones_t[0:1, :],
                    start=(r == 0), stop=(r == 9),
                )

            # ---- persistent intermediates ----
            kt = [data.tile([128, N_TOK], bf16, tag=f"kt{d}", name=f"kt{d}") for d in range(NCH)]
            qt = [data.tile([128, TB], bf16, tag=f"qt{d}", name=f"qt{d}") for d in range(NCH)]
            vpad = [
                data.tile([128, HEADS, DK + 1], bf16, tag=f"vp{j}", name=f"vp{j}")
                for j in range(NJT)
            ]
            att = [
                data.tile([128, TB], bf16, tag=f"att{c}", name=f"att{c}")
                for c in range(NCH)
            ]

            # accumulation c-order (0,2,1,3): chunks 0/2 land first (they
            # lead their DMA rings), so the head chains start ~3.5us before
            # chunks 1/3 arrive instead of stalling on c=1 in order
            CORD = (0, 2, 1, 3)

            def proj_q(d):
                """Q^T d-tile (128 chans = heads 2d, 2d+1) + bias."""
                qp = aux.tile([128, TB], f32, tag="aux", name=f"qp{d}")
                for i, c in enumerate(CORD):
                    nc.tensor.matmul(
                        qp[:], wqd(d, c), xt_sb[c][:, 0:TB],
                        start=(i == 0), stop=(i == NCH - 1),
                    )
                nc.vector.tensor_scalar_add(out=qt[d][:], in0=qp[:], scalar1=bqt[d][:])

            def proj_k(d, jb):
                """K^T d-tile, token block jb + bias."""
                kp = aux.tile([128, TB], f32, tag="aux", name=f"kp{d}_{jb}")
                for i, c in enumerate(CORD):
                    nc.tensor.matmul(
                        kp[:], wkd(d, c), xt_sb[c][:, ts(jb, TB)],
                        start=(i == 0), stop=(i == NCH - 1),
                    )
                nc.vector.tensor_scalar_add(
                    out=kt[d][:, ts(jb, TB)], in0=kp[:], scalar1=bkt[d][:]
                )

            def proj_kq(d):
                proj_q(d)
                for jb in range(N_TOK // TB):
                    proj_k(d, jb)

            def proj_v(j):
                """V j-tile -> padded [128, 8, 65] with ones in column 64.
                The PSUM->SBUF evacuations alternate between ScalarE and
                DVE so neither engine paces pair 0 (GpSimd cannot read
                PSUM, so it can't take these). j=13/15 go to ScalarE so
                the pair-boundary DVE queue (fast-exp 13/15 + the next
                pair's projection bias adds) stays short."""
                vp = aux.tile([128, C], f32, tag="aux", name=f"vpp{j}")
                for c in range(NCH):
                    nc.tensor.matmul(
                        vp[:], xt_sb[c][:, ts(j, 128)], wv[c][:],
                        start=(c == 0), stop=(c == NCH - 1),
                    )
                src = vp[:].rearrange("p (h d) -> p h d", h=HEADS)
                if j % 2 == 0 or j >= 13:
                    nc.scalar.copy(out=vpad[j][:, :, 0:DK], in_=src)
                else:
                    nc.vector.tensor_copy(out=vpad[j][:, :, 0:DK], in_=src)
                nc.vector.memset(vpad[j][:, :, DK : DK + 1], 1.0)

            # ---- main pipeline ----
            # The pair's softmax denominators live in rows 0 and 32 of a
            # [33, TB] dn tile (matmul rhs base partitions must be 0/32/64)
            # so the ln/exp reciprocal runs as one ScalarE call per
            # function. Rows 1-31 are memset to 1.0 ahead of time: the
            # selector matmul contracts over all 33 rcp rows, and ln/exp of
            # stale SBUF garbage can be inf/nan, which survives a 0-weight
            # (0*nan=nan). Row copies read the PSUM accumulators directly.
            def norm_recip(pp, dn):
                """Batched reciprocal of both denominators on ScalarE as
                exp(-ln(x)) — both functions live in one ACT table set, and
                it keeps the slow iterative divide off DVE. fp16 output so
                the broadcast matmul runs at 1 cycle/row (an fp32 rhs costs
                two half-speed PE passes)."""
                lnt = npool.tile([33, TB], f32, tag="lnt", bufs=2, name=f"lnt{pp}")
                nc.scalar.activation(out=lnt[:], in_=dn[:], func=Ln)
                rcp = npool.tile([33, TB], f16, tag="rcp", bufs=2, name=f"rcp{pp}")
                nc.scalar.activation(out=rcp[:], in_=lnt[:], func=Exp, scale=-1.0)
                return rcp

            def norm_apply(pp, asb, rcp):
                """Both heads at once: one selector matmul broadcasts rcp
                rows 0/32 onto out partitions 0-63/64-127, one DVE multiply
                normalizes the whole pair."""
                rb = aux.tile([128, TB], f32, tag="aux", name=f"rb{pp}")
                nc.tensor.matmul(rb[:], sel[:, :], rcp[:, :])
                nc.vector.tensor_mul(out=att[pp][:, :], in0=asb[:, :], in1=rb[:])

            proj_q(0)
            proj_k(0, 0)
            fps = []  # fc PSUM accumulators; t=0/1 filled in pair 3's loop
            prev = None  # previous pair's SBUF accumulator copies
            prev_dn = None
            prev_rcp = None
            for p in range(NCH):  # head pair p = heads 2p, 2p+1
                a0 = apool.tile([DK + 1, TB], f32, tag="a0", name=f"a0_{p}")
                a1 = apool.tile([DK + 1, TB], f32, tag="a1", name=f"a1_{p}")
                fexp_js = FEXP_J[p]

                def attn_v(j, e):
                    nc.tensor.matmul(
                        a0[:], vpad[j][:, 2 * p, :], e[:, 0:TB],
                        start=(j == 0), stop=(j == NJT - 1),
                    )
                    nc.tensor.matmul(
                        a1[:], vpad[j][:, 2 * p + 1, :], e[:, TB : 2 * TB],
                        start=(j == 0), stop=(j == NJT - 1),
                    )

                pend = None  # (j, e) whose attnV is deferred one iteration
                for j in range(NJT):
                    # pair 0: the rest of its K^T spread through the loop
                    if p == 0 and j in (1, 4, 6):
                        proj_k(0, {1: 1, 4: 2, 6: 3}[j])
                    # pair 1's jb1 (the p0 boundary only carries Q+jb0)
                    if p == 1 and j == 1:
                        proj_k(1, 1)
                    # each pair's own late K pieces run early in its own
                    # loop (jb2 feeds sc j>=8, jb3 feeds sc j>=12). Keeping
                    # them out of the boundary keeps the boundary DVE queue
                    # (which gates the next pair's first scores via the
                    # projection bias adds) short.
                    if p >= 1:
                        if j == 4:
                            proj_k(p, 2)
                        elif j == 8:
                            proj_k(p, 3)
                    # next pair's Q + early K mid-pair: their DVE bias adds
                    # drain long before the boundary
                    if 0 < p < NCH - 1:
                        if j == 10:
                            proj_q(p + 1)
                        elif j in (12, 14):
                            proj_k(p + 1, (j - 12) // 2)
                    # the pair's denominator tile: allocate + clear early so
                    # the boundary only does the two row copies (the memset
                    # is ~500ns of DVE that would sit in the critical queue)
                    if j == 13:
                        dn_cur = npool.tile([33, TB], f32, tag="dn", bufs=2, name=f"dn{p}")
                        nc.vector.memset(dn_cur[0:32, :], 1.0)
                    # previous pair's normalization, deferred into this
                    # pair's loop so its reciprocal/broadcast work doesn't
                    # gate PE at the boundary. The last pair normalizes
                    # early (j=0-2) so att[2] is ready for in-loop fc
                    # prefill — real PE work that keeps the clock monitor
                    # from throttling the otherwise projection-free pair 3.
                    if prev is not None:
                        if p < NCH - 1:
                            if j == 3:
                                prev_rcp = norm_recip(p - 1, prev_dn)
                            elif j == 5:
                                norm_apply(p - 1, prev, prev_rcp)
                        else:
                            if j == 0:
                                prev_rcp = norm_recip(p - 1, prev_dn)
                            elif j == 1:
                                norm_apply(p - 1, prev, prev_rcp)
                    # fc prefill for token chunks 0/1 interleaved into the
                    # last pair's loop (aux PSUM slots are free here). These
                    # full-array matmuls also re-warm the clock monitor,
                    # which the half-array sc/attnV mix cannot.
                    if p == NCH - 1 and j in (3, 5, 7, 9, 11, 13):
                        t = int(j >= 9)
                        c = ((j - 3) % 6) // 2
                        if c == 0:
                            fp = aux.tile([128, C], f32, tag="aux", name=f"fp{t}")
                            fps.append(fp)
                            nc.tensor.matmul(
                                fp[:], ones_t[0:1, 0:128], bfct[:],
                                start=True, stop=False,
                            )
                        nc.tensor.matmul(
                            fps[t][:], att[c][:, ts(t, 128)], wf[c][:],
                            start=False, stop=False,
                        )
                    sc = scp.tile([128, 2 * TB], f32, tag="sc", name=f"sc{p}_{j}")
                    nc.tensor.matmul(
                        sc[:, 0:TB], kt[p][0:64, ts(j, 128)], qt[p][0:64, :]
                    )
                    nc.tensor.matmul(
                        sc[:, TB : 2 * TB], kt[p][64:128, ts(j, 128)], qt[p][64:128, :]
                    )
                    e = ep.tile([128, 2 * TB], bf16, tag="e", name=f"e{p}_{j}")
                    if j in fexp_js:
                        # Schraudolph fast-exp on DVE: bf16 bit pattern via
                        # int16 round(x*C1 + C2); error ~3% per weight,
                        # washes out over the 2048-key softmax average
                        nc.vector.tensor_scalar(
                            out=e[:].bitcast(i16), in0=sc[:],
                            scalar1=FEXP_C1, scalar2=FEXP_C2, op0=Mult, op1=Add,
                        )
                    else:
                        nc.scalar.activation(out=e[:], in_=sc[:], func=Exp, scale=0.125)
                    # V projection emitted after scores/exp so a late wv/xt
                    # DMA can't block the PE stream ahead of the scores
                    if p == 0:
                        proj_v(j)
                    # attnV deferred one iteration: the tensor queue is
                    # strict FIFO, so attnV(j) right here would block the
                    # already-queued sc(j+1) behind exp(j). One iteration
                    # of skew keeps PE from ever waiting on the exp stream.
                    if pend is not None:
                        attn_v(*pend)
                    pend = (j, e)
                # pair 1's critical projections lead the boundary: their PE
                # chains fill the exp15 wait and their DVE bias adds (which
                # gate pair 1's first scores) queue ahead of the evacuation
                # traffic
                if p == 0:
                    proj_q(1)
                    proj_k(1, 0)
                attn_v(*pend)
                # denominator rows first (they feed the next pair's recip),
                # then evacuate both accumulators into one [128, TB] SBUF
                # tile (head 2p rows 0-63, head 2p+1 rows 64-127)
                nc.vector.tensor_copy(out=dn_cur[0:1, :], in_=a0[64:65, :])
                nc.vector.tensor_copy(out=dn_cur[32:33, :], in_=a1[64:65, :])
                prev_dn = dn_cur
                asb = npool.tile([128, TB], f32, tag="asb", bufs=2, name=f"asb_{p}")
                nc.vector.tensor_copy(out=asb[0:64, :], in_=a0[0:64, :])
                if p < NCH - 1:
                    nc.scalar.copy(out=asb[64:128, :], in_=a1[0:64, :])
                else:
                    # pair 3: keep ScalarE free for the tail reciprocals
                    nc.vector.tensor_copy(out=asb[64:128, :], in_=a1[0:64, :])
                prev = asb

            # ---- tail: all four fc tiles pre-accumulate bias + the first
            # three chunks (PE filler while the last pair's reciprocals run
            # on ScalarE); only the final chunk waits on normalize(3) ----
            def fc_prefill(t, fp):
                nc.tensor.matmul(
                    fp[:], ones_t[0:1, 0:128], bfct[:], start=True, stop=False
                )
                for c in range(NCH - 1):
                    nc.tensor.matmul(
                        fp[:], att[c][:, ts(t, 128)], wf[c][:],
                        start=False, stop=False,
                    )

            for t in (2, 3):
                # scores pool is draining by now; reuse its slots
                fp = scp.tile([128, C], f32, tag="sc", name=f"fp{t}")
                fc_prefill(t, fp)
                fps.append(fp)
            # last pair's normalization, pipelined per fc token chunk: all
            # four [33, 128] reciprocal chunks run back-to-back on ScalarE
            # first (nothing else sits in that queue), then each chunk's
            # broadcast -> multiply -> fc -> evac -> DMA chain overlaps
            # with the next chunk's. The ot evacuations go to ScalarE
            # (free after the recips) so they never block the DVE
            # multiplies. rb3 lives in the attnV accumulator pool (free
            # after the acopies); each chunk's broadcast writes its own
            # column window of the one PSUM bank.
            rcps = []
            for t in range(NTT):
                lnt_t = npool.tile([33, 128], f32, tag="lnt", bufs=2, name=f"lnt3_{t}")
                nc.scalar.activation(out=lnt_t[:], in_=prev_dn[:, ts(t, 128)], func=Ln)
                rcp_t = npool.tile([33, 128], f16, tag="rcpt", bufs=4, name=f"rcp3_{t}")
                nc.scalar.activation(out=rcp_t[:], in_=lnt_t[:], func=Exp, scale=-1.0)
                rcps.append(rcp_t)
            rb3 = apool.tile([128, TB], f32, tag="a0", name="rb3")
            out_q = [nc.sync, nc.scalar, nc.gpsimd, nc.sync]
            for t in range(NTT):
                nc.tensor.matmul(rb3[:, ts(t, 128)], sel[:, :], rcps[t][:, :])
                nc.vector.tensor_mul(
                    out=att[NCH - 1][:, ts(t, 128)],
                    in0=prev[:, ts(t, 128)],
                    in1=rb3[:, ts(t, 128)],
                )
                nc.tensor.matmul(
                    fps[t][:], att[NCH - 1][:, ts(t, 128)], wf[NCH - 1][:],
                    start=False, stop=True,
                )
                ot = npool.tile([128, C], bf16, tag="ot", bufs=4, name=f"ot{t}")
                nc.scalar.copy(out=ot[:], in_=fps[t][:])
                out_q[t].dma_start(out=out_d[ts(t, 128), :], in_=ot[:])

    _split_multi_waits(nc)
    nc.finalize()
    return nc


def get_nc():
    if "nc" not in _CACHE:
        _install_tile_drain_patch()
        _CACHE["nc"] = _build()
    return _CACHE["nc"]


def make_in_maps(x, Wq, bq, Wk, bk, Wv, bv, Wfc, bfc):
    bf = ml_dtypes.bfloat16
    x = np.asarray(x, np.float32)
    Wq, Wk, Wv, Wfc = (np.asarray(w, np.float32) for w in (Wq, Wk, Wv, Wfc))
    bq, bk, bv, bfc = (np.asarray(v, np.float32) for v in (bq, bk, bv, bfc))

    def interleave(wT):
        # [C, cols] -> [128, NCH*cols] with chunk c at columns [c*cols:...]
        cols = wT.shape[1]
        return np.ascontiguousarray(
            wT.reshape(NCH, 128, cols).transpose(1, 0, 2).reshape(128, NCH * cols)
        )

    def interleave_d(wT):
        # [C, C] -> [128, NCH*C] d-major: block (d, c) at columns
        # [(d*NCH+c)*128 : ...], so the d=0 slice [:, 0:NCH*128] ships as
        # one small early DMA
        return np.ascontiguousarray(
            wT.reshape(NCH, 128, NCH, 128).transpose(1, 2, 0, 3).reshape(128, NCH * C)
        )

    bfc_folded = (Wfc @ bv + bfc).reshape(1, C).astype(bf)
    wqT = interleave_d(np.ascontiguousarray(Wq.T).astype(bf))
    wkT = interleave_d(np.ascontiguousarray(Wk.T).astype(bf))
    wvT = interleave(np.ascontiguousarray(Wv.T).astype(bf))
    wfT = interleave(np.ascontiguousarray(Wfc.T).astype(bf))
    bias_c = np.concatenate(
        [bq.reshape(NCH, 128).T, bk.reshape(NCH, 128).T], axis=1
    ).astype(np.float32)

    in_maps = []
    for core in range(N_CORES):
        b, t = divmod(core, N_TOK // TB)
        XT = np.ascontiguousarray(x[b].reshape(N_TOK, C).T).astype(bf)
        # rotate tokens so this core's block sits at columns 0:TB — the Q
        # projection then reads xt directly (attention is j-permutation
        # invariant, so K/V token order doesn't matter as long as kt and
        # vpad agree, which they do: both derive from this xt)
        perm = np.r_[t * TB : (t + 1) * TB, 0 : t * TB, (t + 1) * TB : N_TOK]
        in_maps.append(
            {
                "xt": np.ascontiguousarray(XT[:, perm]),
                "wqT": wqT,
                "wkT": wkT,
                "wvT": wvT,
                "wfT": wfT,
                "bias": bias_c,
                "bfc": bfc_folded,
            }
        )
    return in_maps


def assemble(outs):
    """outs: list of 8 dicts with 'out' (512, 512) -> (2, 512, 64, 32)."""
    per_batch = [
        np.concatenate([outs[b * 4 + t]["out"] for t in range(4)], axis=0)
        for b in range(B)
    ]
    return np.stack(per_batch).reshape(B, C, 64, 32).astype(np.float32)


def kernel(**inputs):
    from concourse.bass_utils import run_bass_kernel_spmd

    nc = get_nc()
    in_maps = make_in_maps(**inputs)
    res = run_bass_kernel_spmd(nc, in_maps, list(range(N_CORES)))
    return assemble(res.results)



# revision 34
# speedup vs baseline: 1.0056x; 1.0056x over previous
"""MultiHeadChannelAttention Bass kernel for 8 Trainium2 NeuronCores.

Problem (hardcoded shapes): x (2, 512, 64, 32) fp32; Wq/Wk/Wv/Wfc (512, 512);
biases (512,). Reference math per batch b, with X = x[b].reshape(2048, 512):
  Q = X Wq^T + bq ; K = X Wk^T + bk ; V = X Wv^T + bv   (heads of 64 dims)
  out = softmax(QK^T/8) V  (per head), concat heads, @ Wfc^T + bfc

Sharding: 8 cores = 2 batches x 4 token-blocks of 512 tokens. Each core
computes K/V for all 2048 tokens of its batch (4x redundant), Q/attention/fc
only for its 512-token block. No cross-core communication; the host only
slices inputs and concatenates outputs. Tokens are rotated per-core so the
core's own block sits at columns 0:512 of X^T — the Q projection then reads
the same xt tiles as K/V and no separate xq tensor is shipped.

Device layouts (all matmul-friendly, weights pre-transposed on host):
  XT  [512c, 2048t]  = X^T (rotated), one full-width tile per 128-chan chunk
  KT [512, 2048] = (Wk X^T + bk);  QT [512, 512]
  scoresT [j, i] per head via row-tiled K=64 matmul pairs (2 heads/PE pass,
  concurrent in the PE array via distinct row groups)
  exp on ScalarE from 2-bank PSUM; on ~1/4 of tiles (always the last js
  of each pair, so boundary attnVs never wait on ScalarE) a DVE
  Schraudolph fast-exp (bf16-bit-pattern trick, one TENSOR_SCALAR) stands in
  so the exp stream isn't ScalarE-paced; attnV with ones-column (M=65) so the
  softmax denominator falls out of the same matmul; fc consumes attnout^T
  directly. bv is folded into the fc bias on host (softmax rows sum to 1).
  Softmax reciprocal = exp(-ln(x)) on ScalarE, batched [2, 512] per pair,
  broadcast to both head rows with a single fp16 selector matmul (fp32
  matmuls cost two half-speed PE passes). Output leaves as bf16; the host
  casts back to fp32.
"""

import numpy as np
import ml_dtypes

N_CORES = 8
B, C, N_TOK, TB = 2, 512, 2048, 512
HEADS, DK = 8, 64
NCH = C // 128  # channel chunks (4)
NJT = N_TOK // 128  # key-token tiles (16)
NTT = TB // 128  # fc token tiles (4)
HT = N_TOK // 2  # xt column-half (1024)

# Schraudolph fast-exp constants for bf16 bit patterns, including the 0.125
# attention scale: i16 = round(score * (0.125*128/ln2) + (127*128 - 5.5))
FEXP_C1 = 0.125 * 128.0 / float(np.log(2.0))
FEXP_C2 = 127.0 * 128.0 - 5.25
# iterations (per head-pair) whose exp runs on DVE instead of ScalarE.
# j=13/15 are always DVE so the pair's last attnVs never wait on the
# ScalarE exp queue at the boundary funnel.
FEXP_J = {
    0: (13, 15),
    1: (1, 7, 13, 15),
    2: (1, 7, 13, 15),
    3: (1, 3, 5, 9, 11, 13, 15),
}

_CACHE = {}


def _install_tile_drain_patch():
    """The end-of-kernel Tile drain can carry several sem waits; this
    walrus build allows one wait per non-EVSEM instruction. Split the
    waits across a chain of drains."""
    import bass_rust
    from concourse import tile as _tile
    from concourse.vector_clock import ScopedClock

    if getattr(_tile.TileContext, "_drain_patch_installed", False):
        return

    def _patched(self, tick_clock, wait_clock):
        nc = self.nc
        drain_inst = nc.sync.drain()
        wait_clock.add_sem_waits(
            drain_inst.ins, ScopedClock({None: tick_clock.global_clock})
        )
        si = drain_inst.ins.sync_info
        if si is not None and len(si.on_wait) > 1:
            waits = list(si.on_wait)
            drain_inst.ins.sync_info = bass_rust.SyncInfo(
                on_wait=[waits[0]], on_update=list(si.on_update)
            )
            for w in waits[1:]:
                extra = nc.sync.drain()
                extra.ins.sync_info = bass_rust.SyncInfo(on_wait=[w], on_update=[])
        nc.all_engine_barrier()
        assert self.sems is not None
        popped = nc._tile_sem_poison_stack.pop()
        assert popped is self._sem_poison
        nc.clear_and_free_semaphores(list(self.sems.allocated().values()))
        nc.all_engine_barrier()

    _tile.TileContext._drain_and_barrier = _patched
    _tile.TileContext._drain_patch_installed = True


def _split_multi_waits(nc):
    """This walrus build accepts one sync wait per instruction (two on
    EVSEM). Tile can attach two; move extras onto preceding NOPs."""
    import concourse.mybir as mybir

    for f in nc.m.functions:
        for bb in f.blocks:
            out = []
            changed = False
            for ins in bb.instructions:
                si = ins.sync_info
                limit = 2 if isinstance(ins, mybir.InstEventSemaphore) else 1
                if si is not None and len(si.on_wait) > limit:
                    waits = list(si.on_wait)
                    keep = waits[-limit:]
                    for i, w in enumerate(waits[:-limit]):
                        nop = mybir.InstNoOp(
                            name=f"{ins.name}_w{i}",
                            engine=ins.engine,
                            sync_info=mybir.SyncInfo(on_wait=[w], on_update=[]),
                            bass_nofuse=True,
                        )
                        nc.register_instruction(nop, overwrite=True)
                        out.append(nop)
                    ins.sync_info = mybir.SyncInfo(
                        on_wait=keep, on_update=list(si.on_update)
                    )
                    changed = True
                out.append(ins)
            if changed:
                bb.instructions = out


def _build():
    import concourse.bass as bass
    import concourse.mybir as mybir
    import concourse.tile as tile
    from concourse.bass import ts

    dt = mybir.dt
    f32, bf16, i16, f16 = dt.float32, dt.bfloat16, dt.int16, dt.float16
    Exp = mybir.ActivationFunctionType.Exp
    Ln = mybir.ActivationFunctionType.Ln
    Mult, Add = mybir.AluOpType.mult, mybir.AluOpType.add

    nc = bass.Bass()
    # weights are host-interleaved to [128, NCH*cols] so each DMA moves
    # one big per-partition span (large DMA packets) while chunk c still
    # slices out as [:, c*cols : ...] with partition p = channel 128c+p
    xt_d = nc.dram_tensor("xt", [C, N_TOK], bf16, kind="ExternalInput")
    wqT_d = nc.dram_tensor("wqT", [128, NCH * C], bf16, kind="ExternalInput")
    wkT_d = nc.dram_tensor("wkT", [128, NCH * C], bf16, kind="ExternalInput")
    wvT_d = nc.dram_tensor("wvT", [128, NCH * C], bf16, kind="ExternalInput")
    wfT_d = nc.dram_tensor("wfT", [128, NCH * C], bf16, kind="ExternalInput")
    bias_d = nc.dram_tensor("bias", [128, 2 * NCH], f32, kind="ExternalInput")
    bfc_d = nc.dram_tensor("bfc", [1, C], bf16, kind="ExternalInput")
    out_d = nc.dram_tensor("out", [TB, C], bf16, kind="ExternalOutput")

    with tile.TileContext(nc) as tc:
        with (
            tc.tile_pool(name="wp", bufs=1) as wp,
            tc.tile_pool(name="data", bufs=1) as data,
            tc.tile_pool(name="ep", bufs=6) as ep,
            tc.tile_pool(name="np_", bufs=2) as npool,
            tc.tile_pool(name="scp", bufs=2, space=bass.MemorySpace.PSUM) as scp,
            tc.tile_pool(name="ap_", bufs=1, space=bass.MemorySpace.PSUM) as apool,
            tc.tile_pool(name="aux", bufs=2, space=bass.MemorySpace.PSUM) as aux,
        ):
            # ---- constants / weights (merged [128, NCH*cols] tiles).
            # wq/wk are d-major ((d, c) 128-col blocks) so the d=0 pieces
            # that gate the first scores ship as small early DMAs; wv/wf
            # stay c-major (consumed whole-chunk) ----
            wq_all = wp.tile([128, NCH * C], bf16, tag="wq", name="wq_all")
            wk_all = wp.tile([128, NCH * C], bf16, tag="wk", name="wk_all")
            wv_all = wp.tile([128, NCH * C], bf16, tag="wv", name="wv_all")
            wf_all = wp.tile([128, NCH * C], bf16, tag="wf", name="wf_all")

            def wqd(d, c):
                return wq_all[:, ts(d * NCH + c, 128)]

            def wkd(d, c):
                return wk_all[:, ts(d * NCH + c, 128)]

            wv = [wv_all[:, ts(c, C)] for c in range(NCH)]
            wf = [wf_all[:, ts(c, C)] for c in range(NCH)]
            bias_all = wp.tile([128, 2 * NCH], f32, tag="bias", name="bias_all")
            bqt = [bias_all[:, d : d + 1] for d in range(NCH)]
            bkt = [bias_all[:, NCH + d : NCH + d + 1] for d in range(NCH)]
            bfct = wp.tile([1, C], bf16, tag="bfct", name="bfct")
            ones_t = wp.tile([128, TB], bf16, tag="ones", name="ones_t")
            nc.gpsimd.memset(ones_t[:], 1.0)
            ones_f = wp.tile([128, 64], f32, tag="onesf", name="ones_f")
            nc.vector.memset(ones_f[:], 1.0)
            # selector for the denominator broadcast: one fp16 matmul maps
            # rcp row 0 -> out partitions 0-63 and row 32 -> 64-127
            sel = wp.tile([33, 128], f16, tag="sel", name="sel")
            nc.vector.memset(sel[:], 0.0)
            nc.vector.memset(sel[0:1, 0:64], 1.0)
            nc.vector.memset(sel[32:33, 64:128], 1.0)

            # ---- activations in: one full-width tile per channel chunk.
            # Full 4KB DRAM rows: DMA cost is ~27ns per partition-row
            # regardless of row bytes (up to 4KB), so half-width tiles
            # would double the load time for the same data ----
            xt_sb = [
                data.tile([128, N_TOK], bf16, tag=f"xt{c}", name=f"xt{c}")
                for c in range(NCH)
            ]

            # ---- input DMAs over the three issue paths (SP/ACT HWDGE +
            # gpsimd SWDGE). DMA cost is ~flat per 128-partition-row
            # (independent of row bytes up to 4KB), so every tensor ships
            # as ONE whole-tile DMA (column-split pieces would double the
            # row count) and the rings share the ~300GB/s aggregate.
            # Critical set for the first scores: bias + wq + wk + all xt
            # (~896 row-units -> last lands ~17.5us); wv gates the first
            # attnV; wf only matters at the fc prefill (~100us) ----
            nc.sync.dma_start(out=bias_all[:], in_=bias_d[:])
            nc.sync.dma_start(out=xt_sb[0][:], in_=xt_d[ts(0, 128), :])
            nc.sync.dma_start(out=xt_sb[1][:], in_=xt_d[ts(1, 128), :])
            nc.scalar.dma_start(out=wq_all[:], in_=wqT_d[:])
            nc.scalar.dma_start(out=wk_all[:], in_=wkT_d[:])
            nc.gpsimd.dma_start(out=xt_sb[2][:], in_=xt_d[ts(2, 128), :])
            nc.gpsimd.dma_start(out=xt_sb[3][:], in_=xt_d[ts(3, 128), :])
            nc.scalar.dma_start(out=wv_all[:], in_=wvT_d[:])
            nc.gpsimd.dma_start(out=wf_all[:], in_=wfT_d[:])
            nc.sync.dma_start(out=bfct[:], in_=bfc_d[:])

            # trigger the natural_log_exp ACT table load during the DMA
            # window instead of right before the first real exp
            tbl = npool.tile([1, 64], f32, tag="tbl", bufs=1, name="tbl")
            nc.scalar.activation(out=tbl[:], in_=ones_f[0:1, :], func=Ln)

            # PE warmup: one dummy accumulation chain on the ones tile keeps
            # the HAM activity monitor busy through the input-load window so
            # the first real projections run at 2.4 GHz
            warm = aux.tile([128, TB], f32, tag="aux", name="warm")
            for r in range(10):
                nc.tensor.matmul(
                    warm[:], ones_t[0:1, 0:128], ones_t[0:1, :],
                    start=(r == 0), stop=(r == 9),
                )

            # ---- persistent intermediates ----
            kt = [data.tile([128, N_TOK], bf16, tag=f"kt{d}", name=f"kt{d}") for d in range(NCH)]
            qt = [data.tile([128, TB], bf16, tag=f"qt{d}", name=f"qt{d}") for d in range(NCH)]
            vpad = [
                data.tile([128, HEADS, DK + 1], bf16, tag=f"vp{j}", name=f"vp{j}")
                for j in range(NJT)
            ]
            att = [
                data.tile([128, TB], bf16, tag=f"att{c}", name=f"att{c}")
                for c in range(NCH)
            ]

            # accumulation c-order (0,2,1,3): chunks 0/2 land first (they
            # lead their DMA rings), so the head chains start ~3.5us before
            # chunks 1/3 arrive instead of stalling on c=1 in order
            CORD = (0, 2, 1, 3)

            def proj_q(d):
                """Q^T d-tile (128 chans = heads 2d, 2d+1) + bias."""
                qp = aux.tile([128, TB], f32, tag="aux", name=f"qp{d}")
                for i, c in enumerate(CORD):
                    nc.tensor.matmul(
                        qp[:], wqd(d, c), xt_sb[c][:, 0:TB],
                        start=(i == 0), stop=(i == NCH - 1),
                    )
                nc.vector.tensor_scalar_add(out=qt[d][:], in0=qp[:], scalar1=bqt[d][:])

            def proj_k(d, jb):
                """K^T d-tile, token block jb + bias."""
                kp = aux.tile([128, TB], f32, tag="aux", name=f"kp{d}_{jb}")
                for i, c in enumerate(CORD):
                    nc.tensor.matmul(
                        kp[:], wkd(d, c), xt_sb[c][:, ts(jb, TB)],
                        start=(i == 0), stop=(i == NCH - 1),
                    )
                nc.vector.tensor_scalar_add(
                    out=kt[d][:, ts(jb, TB)], in0=kp[:], scalar1=bkt[d][:]
                )

            def proj_kq(d):
                proj_q(d)
                for jb in range(N_TOK // TB):
                    proj_k(d, jb)

            def proj_v(j):
                """V j-tile -> padded [128, 8, 65] with ones in column 64.
                The PSUM->SBUF evacuations alternate between ScalarE and
                DVE so neither engine paces pair 0 (GpSimd cannot read
                PSUM, so it can't take these). j=13/15 go to ScalarE so
                the pair-boundary DVE queue (fast-exp 13/15 + the next
                pair's projection bias adds) stays short."""
                vp = aux.tile([128, C], f32, tag="aux", name=f"vpp{j}")
                for c in range(NCH):
                    nc.tensor.matmul(
                        vp[:], xt_sb[c][:, ts(j, 128)], wv[c][:],
                        start=(c == 0), stop=(c == NCH - 1),
                    )
                src = vp[:].rearrange("p (h d) -> p h d", h=HEADS)
                if j % 2 == 0 or j >= 13:
                    nc.scalar.copy(out=vpad[j][:, :, 0:DK], in_=src)
                else:
                    nc.vector.tensor_copy(out=vpad[j][:, :, 0:DK], in_=src)
                nc.vector.memset(vpad[j][:, :, DK : DK + 1], 1.0)

            # ---- main pipeline ----
            # The pair's softmax denominators live in rows 0 and 32 of a
            # [33, TB] dn tile (matmul rhs base partitions must be 0/32/64)
            # so the ln/exp reciprocal runs as one ScalarE call per
            # function. Rows 1-31 are memset to 1.0 ahead of time: the
            # selector matmul contracts over all 33 rcp rows, and ln/exp of
            # stale SBUF garbage can be inf/nan, which survives a 0-weight
            # (0*nan=nan). Row copies read the PSUM accumulators directly.
            def norm_recip(pp, dn):
                """Batched reciprocal of both denominators on ScalarE as
                exp(-ln(x)) — both functions live in one ACT table set, and
                it keeps the slow iterative divide off DVE. fp16 output so
                the broadcast matmul runs at 1 cycle/row (an fp32 rhs costs
                two half-speed PE passes)."""
                lnt = npool.tile([33, TB], f32, tag="lnt", bufs=2, name=f"lnt{pp}")
                nc.scalar.activation(out=lnt[:], in_=dn[:], func=Ln)
                rcp = npool.tile([33, TB], f16, tag="rcp", bufs=2, name=f"rcp{pp}")
                nc.scalar.activation(out=rcp[:], in_=lnt[:], func=Exp, scale=-1.0)
                return rcp

            def norm_apply(pp, asb, rcp):
                """Both heads at once: one selector matmul broadcasts rcp
                rows 0/32 onto out partitions 0-63/64-127, one DVE multiply
                normalizes the whole pair."""
                rb = aux.tile([128, TB], f32, tag="aux", name=f"rb{pp}")
                nc.tensor.matmul(rb[:], sel[:, :], rcp[:, :])
                nc.vector.tensor_mul(out=att[pp][:, :], in0=asb[:, :], in1=rb[:])

            proj_q(0)
            proj_k(0, 0)
            fps = []  # fc PSUM accumulators; t=0/1 filled in pair 3's loop
            prev = None  # previous pair's SBUF accumulator copies
            prev_dn = None
            prev_rcp = None
            for p in range(NCH):  # head pair p = heads 2p, 2p+1
                a0 = apool.tile([DK + 1, TB], f32, tag="a0", name=f"a0_{p}")
                a1 = apool.tile([DK + 1, TB], f32, tag="a1", name=f"a1_{p}")
                fexp_js = FEXP_J[p]

                def attn_v(j, e):
                    nc.tensor.matmul(
                        a0[:], vpad[j][:, 2 * p, :], e[:, 0:TB],
                        start=(j == 0), stop=(j == NJT - 1),
                    )
                    nc.tensor.matmul(
                        a1[:], vpad[j][:, 2 * p + 1, :], e[:, TB : 2 * TB],
                        start=(j == 0), stop=(j == NJT - 1),
                    )

                pend = None  # (j, e) whose attnV is deferred one iteration
                for j in range(NJT):
                    # pair 0: the rest of its K^T spread through the loop
                    if p == 0 and j in (1, 4, 6):
                        proj_k(0, {1: 1, 4: 2, 6: 3}[j])
                    # pair 1's jb1 (the p0 boundary only carries Q+jb0)
                    if p == 1 and j == 1:
                        proj_k(1, 1)
                    # each pair's own late K pieces run early in its own
                    # loop (jb2 feeds sc j>=8, jb3 feeds sc j>=12). Keeping
                    # them out of the boundary keeps the boundary DVE queue
                    # (which gates the next pair's first scores via the
                    # projection bias adds) short.
                    if p >= 1:
                        if j == 4:
                            proj_k(p, 2)
                        elif j == 8:
                            proj_k(p, 3)
                    # next pair's Q + early K mid-pair: their DVE bias adds
                    # drain long before the boundary
                    if 0 < p < NCH - 1:
                        if j == 10:
                            proj_q(p + 1)
                        elif j in (12, 14):
                            proj_k(p + 1, (j - 12) // 2)
                    # the pair's denominator tile: allocate + clear early so
                    # the boundary only does the two row copies (the memset
                    # is ~500ns of DVE that would sit in the critical queue)
                    if j == 13:
                        dn_cur = npool.tile([33, TB], f32, tag="dn", bufs=2, name=f"dn{p}")
                        nc.vector.memset(dn_cur[0:32, :], 1.0)
                    # previous pair's normalization, deferred into this
                    # pair's loop so its reciprocal/broadcast work doesn't
                    # gate PE at the boundary. The last pair normalizes
                    # early (j=0-2) so att[2] is ready for in-loop fc
                    # prefill — real PE work that keeps the clock monitor
                    # from throttling the otherwise projection-free pair 3.
                    if prev is not None:
                        if p < NCH - 1:
                            if j == 3:
                                prev_rcp = norm_recip(p - 1, prev_dn)
                            elif j == 5:
                                norm_apply(p - 1, prev, prev_rcp)
                        else:
                            if j == 0:
                                prev_rcp = norm_recip(p - 1, prev_dn)
                            elif j == 1:
                                norm_apply(p - 1, prev, prev_rcp)
                    # fc prefill for token chunks 0/1 interleaved into the
                    # last pair's loop (aux PSUM slots are free here). These
                    # full-array matmuls also re-warm the clock monitor,
                    # which the half-array sc/attnV mix cannot.
                    if p == NCH - 1 and j in (3, 5, 7, 9, 11, 13):
                        t = int(j >= 9)
                        c = ((j - 3) % 6) // 2
                        if c == 0:
                            fp = aux.tile([128, C], f32, tag="aux", name=f"fp{t}")
                            fps.append(fp)
                            nc.tensor.matmul(
                                fp[:], ones_t[0:1, 0:128], bfct[:],
                                start=True, stop=False,
                            )
                        nc.tensor.matmul(
                            fps[t][:], att[c][:, ts(t, 128)], wf[c][:],
                            start=False, stop=False,
                        )
                    sc = scp.tile([128, 2 * TB], f32, tag="sc", name=f"sc{p}_{j}")
                    nc.tensor.matmul(
                        sc[:, 0:TB], kt[p][0:64, ts(j, 128)], qt[p][0:64, :]
                    )
                    nc.tensor.matmul(
                        sc[:, TB : 2 * TB], kt[p][64:128, ts(j, 128)], qt[p][64:128, :]
                    )
                    e = ep.tile([128, 2 * TB], bf16, tag="e", name=f"e{p}_{j}")
                    if j in fexp_js:
                        # Schraudolph fast-exp on DVE: bf16 bit pattern via
                        # int16 round(x*C1 + C2); error ~3% per weight,
                        # washes out over the 2048-key softmax average
                        nc.vector.tensor_scalar(
                            out=e[:].bitcast(i16), in0=sc[:],
                            scalar1=FEXP_C1, scalar2=FEXP_C2, op0=Mult, op1=Add,
                        )
                    else:
                        nc.scalar.activation(out=e[:], in_=sc[:], func=Exp, scale=0.125)
                    # V projection emitted after scores/exp so a late wv/xt
                    # DMA can't block the PE stream ahead of the scores
                    if p == 0:
                        proj_v(j)
                    # attnV deferred one iteration: the tensor queue is
                    # strict FIFO, so attnV(j) right here would block the
                    # already-queued sc(j+1) behind exp(j). One iteration
                    # of skew keeps PE from ever waiting on the exp stream.
                    if pend is not None:
                        attn_v(*pend)
                    pend = (j, e)
                # pair 1's critical projections lead the boundary: their PE
                # chains fill the exp15 wait and their DVE bias adds (which
                # gate pair 1's first scores) queue ahead of the evacuation
                # traffic
                if p == 0:
                    proj_q(1)
                    proj_k(1, 0)
                attn_v(*pend)
                # denominator rows first (they feed the next pair's recip),
                # then evacuate both accumulators into one [128, TB] SBUF
                # tile (head 2p rows 0-63, head 2p+1 rows 64-127)
                nc.vector.tensor_copy(out=dn_cur[0:1, :], in_=a0[64:65, :])
                nc.vector.tensor_copy(out=dn_cur[32:33, :], in_=a1[64:65, :])
                prev_dn = dn_cur
                if p < NCH - 1:
                    asb = npool.tile([128, TB], f32, tag="asb", bufs=2, name=f"asb_{p}")
                    nc.vector.tensor_copy(out=asb[0:64, :], in_=a0[0:64, :])
                    nc.scalar.copy(out=asb[64:128, :], in_=a1[0:64, :])
                    prev = asb
                else:
                    # pair 3 skips the SBUF evacuation entirely: the tail
                    # multiplies read the PSUM accumulators directly, so
                    # the critical chain is just dn -> recip -> broadcast
                    prev = (a0, a1)

            # ---- tail: all four fc tiles pre-accumulate bias + the first
            # three chunks (PE filler while the last pair's reciprocals run
            # on ScalarE); only the final chunk waits on normalize(3) ----
            def fc_prefill(t, fp):
                nc.tensor.matmul(
                    fp[:], ones_t[0:1, 0:128], bfct[:], start=True, stop=False
                )
                for c in range(NCH - 1):
                    nc.tensor.matmul(
                        fp[:], att[c][:, ts(t, 128)], wf[c][:],
                        start=False, stop=False,
                    )

            # fp2/fp3 share ONE scores-pool slot (each only needs half its
            # 2-bank width), leaving the other slot free for rb3 below
            fp23 = scp.tile([128, 2 * TB], f32, tag="sc", name="fp23")
            for t in (2, 3):
                fp = fp23[:, ts(t - 2, C)]
                fc_prefill(t, fp)
                fps.append(fp)
            # last pair's normalization, pipelined per fc token chunk: all
            # four [33, 128] reciprocal chunks run back-to-back on ScalarE
            # first (nothing else sits in that queue), then each chunk's
            # broadcast -> multiply -> fc -> evac -> DMA chain overlaps
            # with the next chunk's. The ot evacuations go to ScalarE
            # (free after the recips) so they never block the DVE
            # multiplies. rb3 lives in the attnV accumulator pool (free
            # after the acopies); each chunk's broadcast writes its own
            # column window of the one PSUM bank.
            rcps = []
            for t in range(NTT):
                lnt_t = npool.tile([33, 128], f32, tag="lnt", bufs=2, name=f"lnt3_{t}")
                nc.scalar.activation(out=lnt_t[:], in_=prev_dn[:, ts(t, 128)], func=Ln)
                rcp_t = npool.tile([33, 128], f16, tag="rcpt", bufs=4, name=f"rcp3_{t}")
                nc.scalar.activation(out=rcp_t[:], in_=lnt_t[:], func=Exp, scale=-1.0)
                rcps.append(rcp_t)
            a0f, a1f = prev
            rb3 = scp.tile([128, TB], f32, tag="sc", name="rb3")
            rbs = npool.tile([128, TB], f32, tag="asb", bufs=2, name="rbs")
            out_q = [nc.sync, nc.scalar, nc.gpsimd, nc.sync]
            for t in range(NTT):
                nc.tensor.matmul(rb3[:, ts(t, 128)], sel[:, :], rcps[t][:, :])
                # DVE can read only one PSUM operand, and the a0/a1
                # accumulators stay in PSUM — bounce the broadcast chunk
                # through SBUF on ScalarE (idle after the recips)
                nc.scalar.copy(out=rbs[:, ts(t, 128)], in_=rb3[:, ts(t, 128)])
                nc.vector.tensor_mul(
                    out=att[NCH - 1][0:64, ts(t, 128)],
                    in0=a0f[0:64, ts(t, 128)],
                    in1=rbs[0:64, ts(t, 128)],
                )
                nc.vector.tensor_mul(
                    out=att[NCH - 1][64:128, ts(t, 128)],
                    in0=a1f[0:64, ts(t, 128)],
                    in1=rbs[64:128, ts(t, 128)],
                )
                nc.tensor.matmul(
                    fps[t][:], att[NCH - 1][:, ts(t, 128)], wf[NCH - 1][:],
                    start=False, stop=True,
                )
                ot = npool.tile([128, C], bf16, tag="ot", bufs=4, name=f"ot{t}")
                nc.scalar.copy(out=ot[:], in_=fps[t][:])
                out_q[t].dma_start(out=out_d[ts(t, 128), :], in_=ot[:])

    _split_multi_waits(nc)
    nc.finalize()
    return nc


def get_nc():
    if "nc" not in _CACHE:
        _install_tile_drain_patch()
        _CACHE["nc"] = _build()
    return _CACHE["nc"]


def make_in_maps(x, Wq, bq, Wk, bk, Wv, bv, Wfc, bfc):
    bf = ml_dtypes.bfloat16
    x = np.asarray(x, np.float32)
    Wq, Wk, Wv, Wfc = (np.asarray(w, np.float32) for w in (Wq, Wk, Wv, Wfc))
    bq, bk, bv, bfc = (np.asarray(v, np.float32) for v in (bq, bk, bv, bfc))

    def interleave(wT):
        # [C, cols] -> [128, NCH*cols] with chunk c at columns [c*cols:...]
        cols = wT.shape[1]
        return np.ascontiguousarray(
            wT.reshape(NCH, 128, cols).transpose(1, 0, 2).reshape(128, NCH * cols)
        )

    def interleave_d(wT):
        # [C, C] -> [128, NCH*C] d-major: block (d, c) at columns
        # [(d*NCH+c)*128 : ...], so the d=0 slice [:, 0:NCH*128] ships as
        # one small early DMA
        return np.ascontiguousarray(
            wT.reshape(NCH, 128, NCH, 128).transpose(1, 2, 0, 3).reshape(128, NCH * C)
        )

    bfc_folded = (Wfc @ bv + bfc).reshape(1, C).astype(bf)
    wqT = interleave_d(np.ascontiguousarray(Wq.T).astype(bf))
    wkT = interleave_d(np.ascontiguousarray(Wk.T).astype(bf))
    wvT = interleave(np.ascontiguousarray(Wv.T).astype(bf))
    wfT = interleave(np.ascontiguousarray(Wfc.T).astype(bf))
    bias_c = np.concatenate(
        [bq.reshape(NCH, 128).T, bk.reshape(NCH, 128).T], axis=1
    ).astype(np.float32)

    in_maps = []
    for core in range(N_CORES):
        b, t = divmod(core, N_TOK // TB)
        XT = np.ascontiguousarray(x[b].reshape(N_TOK, C).T).astype(bf)
        # rotate tokens so this core's block sits at columns 0:TB — the Q
        # projection then reads xt directly (attention is j-permutation
        # invariant, so K/V token order doesn't matter as long as kt and
        # vpad agree, which they do: both derive from this xt)
        perm = np.r_[t * TB : (t + 1) * TB, 0 : t * TB, (t + 1) * TB : N_TOK]
        in_maps.append(
            {
                "xt": np.ascontiguousarray(XT[:, perm]),
                "wqT": wqT,
                "wkT": wkT,
                "wvT": wvT,
                "wfT": wfT,
                "bias": bias_c,
                "bfc": bfc_folded,
            }
        )
    return in_maps


def assemble(outs):
    """outs: list of 8 dicts with 'out' (512, 512) -> (2, 512, 64, 32)."""
    per_batch = [
        np.concatenate([outs[b * 4 + t]["out"] for t in range(4)], axis=0)
        for b in range(B)
    ]
    return np.stack(per_batch).reshape(B, C, 64, 32).astype(np.float32)


def kernel(**inputs):
    from concourse.bass_utils import run_bass_kernel_spmd

    nc = get_nc()
    in_maps = make_in_maps(**inputs)
    res = run_bass_kernel_spmd(nc, in_maps, list(range(N_CORES)))
    return assemble(res.results)



# revision 49
# speedup vs baseline: 1.0129x; 1.0072x over previous
"""MultiHeadChannelAttention Bass kernel for 8 Trainium2 NeuronCores.

Problem (hardcoded shapes): x (2, 512, 64, 32) fp32; Wq/Wk/Wv/Wfc (512, 512);
biases (512,). Reference math per batch b, with X = x[b].reshape(2048, 512):
  Q = X Wq^T + bq ; K = X Wk^T + bk ; V = X Wv^T + bv   (heads of 64 dims)
  out = softmax(QK^T/8) V  (per head), concat heads, @ Wfc^T + bfc

Sharding: 8 cores = 2 batches x 4 token-blocks of 512 tokens. Each core
computes K/V for all 2048 tokens of its batch (4x redundant), Q/attention/fc
only for its 512-token block. No cross-core communication; the host only
slices inputs and concatenates outputs. Tokens are rotated per-core so the
core's own block sits at columns 0:512 of X^T — the Q projection then reads
the same xt tiles as K/V and no separate xq tensor is shipped.

Device layouts (all matmul-friendly, weights pre-transposed on host):
  XT  [512c, 2048t]  = X^T (rotated), one full-width tile per 128-chan chunk
  KT [512, 2048] = (Wk X^T + bk);  QT [512, 512]
  scoresT [j, i] per head via row-tiled K=64 matmul pairs (2 heads/PE pass,
  concurrent in the PE array via distinct row groups)
  exp on ScalarE from 2-bank PSUM; on ~1/4 of tiles (always the last js
  of each pair, so boundary attnVs never wait on ScalarE) a DVE
  Schraudolph fast-exp (bf16-bit-pattern trick, one TENSOR_SCALAR) stands in
  so the exp stream isn't ScalarE-paced; attnV with ones-column (M=65) so the
  softmax denominator falls out of the same matmul; fc consumes attnout^T
  directly. bv is folded into the fc bias on host (softmax rows sum to 1).
  Softmax reciprocal = exp(-ln(x)) on ScalarE, batched [2, 512] per pair,
  broadcast to both head rows with a single fp16 selector matmul (fp32
  matmuls cost two half-speed PE passes). Output leaves as bf16; the host
  casts back to fp32.
"""

import numpy as np
import ml_dtypes

N_CORES = 8
B, C, N_TOK, TB = 2, 512, 2048, 512
HEADS, DK = 8, 64
NCH = C // 128  # channel chunks (4)
NJT = N_TOK // 128  # key-token tiles (16)
NTT = TB // 128  # fc token tiles (4)
HT = N_TOK // 2  # xt column-half (1024)

# Schraudolph fast-exp constants for bf16 bit patterns, including the 0.125
# attention scale: i16 = round(score * (0.125*128/ln2) + (127*128 - 5.5))
FEXP_C1 = 0.125 * 128.0 / float(np.log(2.0))
FEXP_C2 = 127.0 * 128.0 - 5.25
# iterations (per head-pair) whose exp runs on DVE instead of ScalarE.
# j=13/15 are always DVE so the pair's last attnVs never wait on the
# ScalarE exp queue at the boundary funnel.
FEXP_J = {
    0: (13, 15),
    1: (1, 7, 13, 15),
    2: (1, 7, 13, 15),
    # pair 3's j15 stays on ScalarE: the tail reciprocals queue right
    # behind it there, while the DVE fast-exp would sit behind the whole
    # j13/j15 DVE backlog and delay the last attnV
    3: (1, 3, 5, 7, 9, 11, 13),
}

_CACHE = {}


def _install_tile_drain_patch():
    """The end-of-kernel Tile drain can carry several sem waits; this
    walrus build allows one wait per non-EVSEM instruction. Split the
    waits across a chain of drains."""
    import bass_rust
    from concourse import tile as _tile
    from concourse.vector_clock import ScopedClock

    if getattr(_tile.TileContext, "_drain_patch_installed", False):
        return

    def _patched(self, tick_clock, wait_clock):
        nc = self.nc
        drain_inst = nc.sync.drain()
        wait_clock.add_sem_waits(
            drain_inst.ins, ScopedClock({None: tick_clock.global_clock})
        )
        si = drain_inst.ins.sync_info
        if si is not None and len(si.on_wait) > 1:
            waits = list(si.on_wait)
            drain_inst.ins.sync_info = bass_rust.SyncInfo(
                on_wait=[waits[0]], on_update=list(si.on_update)
            )
            for w in waits[1:]:
                extra = nc.sync.drain()
                extra.ins.sync_info = bass_rust.SyncInfo(on_wait=[w], on_update=[])
        nc.all_engine_barrier()
        assert self.sems is not None
        popped = nc._tile_sem_poison_stack.pop()
        assert popped is self._sem_poison
        nc.clear_and_free_semaphores(list(self.sems.allocated().values()))
        nc.all_engine_barrier()

    _tile.TileContext._drain_and_barrier = _patched
    _tile.TileContext._drain_patch_installed = True


def _split_multi_waits(nc):
    """This walrus build accepts one sync wait per instruction (two on
    EVSEM). Tile can attach two; move extras onto preceding NOPs."""
    import concourse.mybir as mybir

    for f in nc.m.functions:
        for bb in f.blocks:
            out = []
            changed = False
            for ins in bb.instructions:
                si = ins.sync_info
                limit = 2 if isinstance(ins, mybir.InstEventSemaphore) else 1
                if si is not None and len(si.on_wait) > limit:
                    waits = list(si.on_wait)
                    keep = waits[-limit:]
                    for i, w in enumerate(waits[:-limit]):
                        nop = mybir.InstNoOp(
                            name=f"{ins.name}_w{i}",
                            engine=ins.engine,
                            sync_info=mybir.SyncInfo(on_wait=[w], on_update=[]),
                            bass_nofuse=True,
                        )
                        nc.register_instruction(nop, overwrite=True)
                        out.append(nop)
                    ins.sync_info = mybir.SyncInfo(
                        on_wait=keep, on_update=list(si.on_update)
                    )
                    changed = True
                out.append(ins)
            if changed:
                bb.instructions = out


def _build():
    import concourse.bass as bass
    import concourse.mybir as mybir
    import concourse.tile as tile
    from concourse.bass import ts

    dt = mybir.dt
    f32, bf16, i16, f16 = dt.float32, dt.bfloat16, dt.int16, dt.float16
    Exp = mybir.ActivationFunctionType.Exp
    Ln = mybir.ActivationFunctionType.Ln
    Mult, Add = mybir.AluOpType.mult, mybir.AluOpType.add

    nc = bass.Bass()
    # weights are host-interleaved to [128, NCH*cols] so each DMA moves
    # one big per-partition span (large DMA packets) while chunk c still
    # slices out as [:, c*cols : ...] with partition p = channel 128c+p
    # wq carries the q/k biases as 16 extra bf16 columns after its d0
    # block — a separate [128, 8] bias tensor would cost a full 128
    # partition-row DMA on the critical path for 32 bytes/row
    xt_d = nc.dram_tensor("xt", [C, N_TOK], bf16, kind="ExternalInput")
    wqT_d = nc.dram_tensor("wqT", [128, NCH * C + 8], bf16, kind="ExternalInput")
    wkT_d = nc.dram_tensor("wkT", [128, NCH * C], bf16, kind="ExternalInput")
    wvT_d = nc.dram_tensor("wvT", [128, NCH * C], bf16, kind="ExternalInput")
    wfT_d = nc.dram_tensor("wfT", [128, NCH * C], bf16, kind="ExternalInput")
    bfc_d = nc.dram_tensor("bfc", [1, C], bf16, kind="ExternalInput")
    out_d = nc.dram_tensor("out", [TB, C], bf16, kind="ExternalOutput")

    with tile.TileContext(nc) as tc:
        with (
            tc.tile_pool(name="wp", bufs=1) as wp,
            tc.tile_pool(name="data", bufs=1) as data,
            tc.tile_pool(name="ep", bufs=6) as ep,
            tc.tile_pool(name="np_", bufs=2) as npool,
            tc.tile_pool(name="scp", bufs=2, space=bass.MemorySpace.PSUM) as scp,
            tc.tile_pool(name="ap_", bufs=1, space=bass.MemorySpace.PSUM) as apool,
            tc.tile_pool(name="aux", bufs=2, space=bass.MemorySpace.PSUM) as aux,
        ):
            # ---- constants / weights (merged [128, NCH*cols] tiles).
            # wq/wk are d-major ((d, c) 128-col blocks) so the d=0 pieces
            # that gate the first scores ship as small early DMAs; wv/wf
            # stay c-major (consumed whole-chunk). wq's layout is
            # [d0 block | 8 bias cols | d1-3 blocks] ----
            wq_all = wp.tile([128, NCH * C + 8], bf16, tag="wq", name="wq_all")
            wk_all = wp.tile([128, NCH * C], bf16, tag="wk", name="wk_all")
            wv_all = wp.tile([128, NCH * C], bf16, tag="wv", name="wv_all")
            wf_all = wp.tile([128, NCH * C], bf16, tag="wf", name="wf_all")

            def wqd(d, c):
                off = (d * NCH + c) * 128 + (8 if d >= 1 else 0)
                return wq_all[:, off : off + 128]

            def wkd(d, c):
                return wk_all[:, ts(d * NCH + c, 128)]

            wv = [wv_all[:, ts(c, C)] for c in range(NCH)]
            wf = [wf_all[:, ts(c, C)] for c in range(NCH)]
            # biases ride in wq's d0 DMA as bf16; cast once to f32 for the
            # DVE tensor_scalar adds
            bias_f32 = wp.tile([128, 8], f32, tag="bias", name="bias_f32")
            bqt = [bias_f32[:, d : d + 1] for d in range(NCH)]
            bkt = [bias_f32[:, NCH + d : NCH + d + 1] for d in range(NCH)]
            bfct = wp.tile([1, C], bf16, tag="bfct", name="bfct")
            ones_t = wp.tile([128, TB], bf16, tag="ones", name="ones_t")
            nc.gpsimd.memset(ones_t[:], 1.0)
            ones_f = wp.tile([128, 64], f32, tag="onesf", name="ones_f")
            nc.vector.memset(ones_f[:], 1.0)
            # selector for the denominator broadcast: one fp16 matmul maps
            # rcp row 0 -> out partitions 0-63 and row 32 -> 64-127
            sel = wp.tile([33, 128], f16, tag="sel", name="sel")
            nc.vector.memset(sel[:], 0.0)
            nc.vector.memset(sel[0:1, 0:64], 1.0)
            nc.vector.memset(sel[32:33, 64:128], 1.0)

            # ---- activations in: two column-half tiles per channel chunk.
            # DMA cost is ~flat per 128-partition-row regardless of row
            # bytes, so halves don't load faster overall — but they halve
            # the CRITICAL subset (first scores need only the first-half
            # tokens), and the second halves overlap into pair 0's loop ----
            xta = [
                data.tile([128, HT], bf16, tag=f"xta{c}", name=f"xta{c}")
                for c in range(NCH)
            ]
            xtb = [
                data.tile([128, HT], bf16, tag=f"xtb{c}", name=f"xtb{c}")
                for c in range(NCH)
            ]

            def xk(jb):  # xt tile list + column block for K token block jb
                return (xta if jb < 2 else xtb), jb % 2

            def xv(j):  # xt tile list + column tile for V token tile j
                return (xta if j < 8 else xtb), j % 8

            # ---- input DMAs over the three issue paths (SP/ACT HWDGE +
            # gpsimd SWDGE). Critical set for the first scores: wq-d0
            # (with the biases riding along) + wk-d0 + the four xta halves
            # (~768 partition-rows -> last lands ~16us); wv gates the
            # first attnV; xtb from pair-0 j~6; wq/wk d1-3 from the pair-0
            # boundary; wf only at the fc prefill (~100us) ----
            NB = NCH * 128  # d0 block columns
            nc.scalar.dma_start(out=wq_all[:, 0 : NB + 8], in_=wqT_d[:, 0 : NB + 8])
            nc.scalar.dma_start(out=wk_all[:, 0:NB], in_=wkT_d[:, 0:NB])
            nc.sync.dma_start(out=xta[0][:], in_=xt_d[ts(0, 128), 0:HT])
            nc.gpsimd.dma_start(out=xta[2][:], in_=xt_d[ts(2, 128), 0:HT])
            nc.sync.dma_start(out=xta[1][:], in_=xt_d[ts(1, 128), 0:HT])
            nc.gpsimd.dma_start(out=xta[3][:], in_=xt_d[ts(3, 128), 0:HT])
            nc.scalar.dma_start(out=wv_all[:], in_=wvT_d[:])
            nc.sync.dma_start(out=xtb[0][:], in_=xt_d[ts(0, 128), HT:N_TOK])
            nc.gpsimd.dma_start(out=xtb[2][:], in_=xt_d[ts(2, 128), HT:N_TOK])
            nc.sync.dma_start(out=xtb[1][:], in_=xt_d[ts(1, 128), HT:N_TOK])
            nc.gpsimd.dma_start(out=xtb[3][:], in_=xt_d[ts(3, 128), HT:N_TOK])
            nc.scalar.dma_start(
                out=wq_all[:, NB + 8 :], in_=wqT_d[:, NB + 8 :]
            )
            nc.scalar.dma_start(out=wk_all[:, NB:], in_=wkT_d[:, NB:])
            nc.gpsimd.dma_start(out=wf_all[:], in_=wfT_d[:])
            nc.sync.dma_start(out=bfct[:], in_=bfc_d[:])
            # f32 copy of the biases for the DVE tensor_scalar adds
            nc.vector.tensor_copy(out=bias_f32[:], in_=wq_all[:, NB : NB + 8])

            # trigger the natural_log_exp ACT table load during the DMA
            # window instead of right before the first real exp
            tbl = npool.tile([1, 64], f32, tag="tbl", bufs=1, name="tbl")
            nc.scalar.activation(out=tbl[:], in_=ones_f[0:1, :], func=Ln)

            # PE warmup: one dummy accumulation chain on the ones tile keeps
            # the HAM activity monitor busy through the input-load window so
            # the first real projections run at 2.4 GHz
            warm = aux.tile([128, TB], f32, tag="aux", name="warm")
            for r in range(10):
                nc.tensor.matmul(
                    warm[:], ones_t[0:1, 0:128], ones_t[0:1, :],
                    start=(r == 0), stop=(r == 9),
                )

            # ---- persistent intermediates ----
            kt = [data.tile([128, N_TOK], bf16, tag=f"kt{d}", name=f"kt{d}") for d in range(NCH)]
            qt = [data.tile([128, TB], bf16, tag=f"qt{d}", name=f"qt{d}") for d in range(NCH)]
            vpad = [
                data.tile([128, HEADS, DK + 1], bf16, tag=f"vp{j}", name=f"vp{j}")
                for j in range(NJT)
            ]
            att = [
                data.tile([128, TB], bf16, tag=f"att{c}", name=f"att{c}")
                for c in range(NCH)
            ]

            # accumulation c-order (0,2,1,3): chunks 0/2 land first (they
            # lead their DMA rings), so the head chains start ~3.5us before
            # chunks 1/3 arrive instead of stalling on c=1 in order
            CORD = (0, 2, 1, 3)

            def proj_q(d):
                """Q^T d-tile (128 chans = heads 2d, 2d+1) + bias."""
                qp = aux.tile([128, TB], f32, tag="aux", name=f"qp{d}")
                for i, c in enumerate(CORD):
                    nc.tensor.matmul(
                        qp[:], wqd(d, c), xta[c][:, 0:TB],
                        start=(i == 0), stop=(i == NCH - 1),
                    )
                nc.vector.tensor_scalar_add(out=qt[d][:], in0=qp[:], scalar1=bqt[d][:])

            def proj_k(d, jb):
                """K^T d-tile, token block jb + bias."""
                xt_half, hb = xk(jb)
                kp = aux.tile([128, TB], f32, tag="aux", name=f"kp{d}_{jb}")
                for i, c in enumerate(CORD):
                    nc.tensor.matmul(
                        kp[:], wkd(d, c), xt_half[c][:, ts(hb, TB)],
                        start=(i == 0), stop=(i == NCH - 1),
                    )
                nc.vector.tensor_scalar_add(
                    out=kt[d][:, ts(jb, TB)], in0=kp[:], scalar1=bkt[d][:]
                )

            def proj_qk0():
                """Fused first Q + K-jb0 chains, c-interleaved: both read
                the same xta tiles, and interleaving lets the c=0/2 pieces
                of BOTH chains run before xta1/xta3 land instead of the K
                chain queueing behind the stalled Q chain."""
                qp = aux.tile([128, TB], f32, tag="aux", name="qp0")
                kp = aux.tile([128, TB], f32, tag="aux", name="kp0_0")
                for i, c in enumerate(CORD):
                    nc.tensor.matmul(
                        qp[:], wqd(0, c), xta[c][:, 0:TB],
                        start=(i == 0), stop=(i == NCH - 1),
                    )
                    nc.tensor.matmul(
                        kp[:], wkd(0, c), xta[c][:, 0:TB],
                        start=(i == 0), stop=(i == NCH - 1),
                    )
                nc.vector.tensor_scalar_add(out=qt[0][:], in0=qp[:], scalar1=bqt[0][:])
                nc.vector.tensor_scalar_add(
                    out=kt[0][:, 0:TB], in0=kp[:], scalar1=bkt[0][:]
                )

            def proj_kq(d):
                proj_q(d)
                for jb in range(N_TOK // TB):
                    proj_k(d, jb)

            def proj_v(j):
                """V j-tile -> padded [128, 8, 65] with ones in column 64.
                The PSUM->SBUF evacuations alternate between ScalarE and
                DVE so neither engine paces pair 0 (GpSimd cannot read
                PSUM, so it can't take these). j=13/15 go to ScalarE so
                the pair-boundary DVE queue (fast-exp 13/15 + the next
                pair's projection bias adds) stays short."""
                xt_half, hj = xv(j)
                vp = aux.tile([128, C], f32, tag="aux", name=f"vpp{j}")
                for c in range(NCH):
                    nc.tensor.matmul(
                        vp[:], xt_half[c][:, ts(hj, 128)], wv[c][:],
                        start=(c == 0), stop=(c == NCH - 1),
                    )
                src = vp[:].rearrange("p (h d) -> p h d", h=HEADS)
                if j % 2 == 0 or j >= 13:
                    nc.scalar.copy(out=vpad[j][:, :, 0:DK], in_=src)
                else:
                    nc.vector.tensor_copy(out=vpad[j][:, :, 0:DK], in_=src)
                nc.vector.memset(vpad[j][:, :, DK : DK + 1], 1.0)

            # ---- main pipeline ----
            # The pair's softmax denominators live in rows 0 and 32 of a
            # [33, TB] dn tile (matmul rhs base partitions must be 0/32/64)
            # so the ln/exp reciprocal runs as one ScalarE call per
            # function. Rows 1-31 are memset to 1.0 ahead of time: the
            # selector matmul contracts over all 33 rcp rows, and ln/exp of
            # stale SBUF garbage can be inf/nan, which survives a 0-weight
            # (0*nan=nan). Row copies read the PSUM accumulators directly.
            def norm_recip(pp, dn):
                """Batched reciprocal of both denominators on ScalarE as
                exp(-ln(x)) — both functions live in one ACT table set, and
                it keeps the slow iterative divide off DVE. fp16 output so
                the broadcast matmul runs at 1 cycle/row (an fp32 rhs costs
                two half-speed PE passes)."""
                lnt = npool.tile([33, TB], f32, tag="lnt", bufs=2, name=f"lnt{pp}")
                nc.scalar.activation(out=lnt[:], in_=dn[:], func=Ln)
                rcp = npool.tile([33, TB], f16, tag="rcp", bufs=2, name=f"rcp{pp}")
                nc.scalar.activation(out=rcp[:], in_=lnt[:], func=Exp, scale=-1.0)
                return rcp

            def norm_apply(pp, asb, rcp):
                """Both heads at once: one selector matmul broadcasts rcp
                rows 0/32 onto out partitions 0-63/64-127, one DVE multiply
                normalizes the whole pair."""
                rb = aux.tile([128, TB], f32, tag="aux", name=f"rb{pp}")
                nc.tensor.matmul(rb[:], sel[:, :], rcp[:, :])
                nc.vector.tensor_mul(out=att[pp][:, :], in0=asb[:, :], in1=rb[:])

            proj_qk0()
            fps = []  # fc PSUM accumulators; t=0/1 filled in pair 3's loop
            prev = None  # previous pair's SBUF accumulator copies
            prev_dn = None
            prev_rcp = None
            for p in range(NCH):  # head pair p = heads 2p, 2p+1
                a0 = apool.tile([DK + 1, TB], f32, tag="a0", name=f"a0_{p}")
                a1 = apool.tile([DK + 1, TB], f32, tag="a1", name=f"a1_{p}")
                fexp_js = FEXP_J[p]

                def attn_v(j, e):
                    nc.tensor.matmul(
                        a0[:], vpad[j][:, 2 * p, :], e[:, 0:TB],
                        start=(j == 0), stop=(j == NJT - 1),
                    )
                    nc.tensor.matmul(
                        a1[:], vpad[j][:, 2 * p + 1, :], e[:, TB : 2 * TB],
                        start=(j == 0), stop=(j == NJT - 1),
                    )

                pend = None  # (j, e) whose attnV is deferred one iteration
                for j in range(NJT):
                    # pair 0: the rest of its K^T spread through the loop
                    # (jb2/3 need the xtb halves, which land ~23us; the
                    # loop reaches j=6 at ~27us)
                    if p == 0 and j in (1, 6, 9):
                        proj_k(0, {1: 1, 6: 2, 9: 3}[j])
                    # pair 1's jb1 (the p0 boundary only carries Q+jb0)
                    if p == 1 and j == 1:
                        proj_k(1, 1)
                    # each pair's own late K pieces run early in its own
                    # loop (jb2 feeds sc j>=8, jb3 feeds sc j>=12). Keeping
                    # them out of the boundary keeps the boundary DVE queue
                    # (which gates the next pair's first scores via the
                    # projection bias adds) short.
                    if p >= 1:
                        if j == 4:
                            proj_k(p, 2)
                        elif j == 8:
                            proj_k(p, 3)
                    # next pair's Q + early K mid-pair: their DVE bias adds
                    # drain long before the boundary
                    if 0 < p < NCH - 1:
                        if j == 10:
                            proj_q(p + 1)
                        elif j in (12, 14):
                            proj_k(p + 1, (j - 12) // 2)
                    # the pair's denominator tile: allocate + clear early so
                    # the boundary only does the two row copies (the memset
                    # is ~500ns of DVE that would sit in the critical queue)
                    if j == 13:
                        dn_cur = npool.tile([33, TB], f32, tag="dn", bufs=2, name=f"dn{p}")
                        nc.vector.memset(dn_cur[0:32, :], 1.0)
                    # previous pair's normalization, deferred into this
                    # pair's loop so its reciprocal/broadcast work doesn't
                    # gate PE at the boundary. The last pair normalizes
                    # early (j=0-2) so att[2] is ready for in-loop fc
                    # prefill — real PE work that keeps the clock monitor
                    # from throttling the otherwise projection-free pair 3.
                    if prev is not None:
                        if p < NCH - 1:
                            if j == 3:
                                prev_rcp = norm_recip(p - 1, prev_dn)
                            elif j == 5:
                                norm_apply(p - 1, prev, prev_rcp)
                        else:
                            if j == 0:
                                prev_rcp = norm_recip(p - 1, prev_dn)
                            elif j == 1:
                                norm_apply(p - 1, prev, prev_rcp)
                    # fc prefill for token chunks 0/1 interleaved into the
                    # last pair's loop (aux PSUM slots are free here). These
                    # full-array matmuls also re-warm the clock monitor,
                    # which the half-array sc/attnV mix cannot.
                    if p == NCH - 1 and j in (3, 5, 7, 9, 11, 13):
                        t = int(j >= 9)
                        c = ((j - 3) % 6) // 2
                        if c == 0:
                            fp = aux.tile([128, C], f32, tag="aux", name=f"fp{t}")
                            fps.append(fp)
                            nc.tensor.matmul(
                                fp[:], ones_t[0:1, 0:128], bfct[:],
                                start=True, stop=False,
                            )
                        nc.tensor.matmul(
                            fps[t][:], att[c][:, ts(t, 128)], wf[c][:],
                            start=False, stop=False,
                        )
                    sc = scp.tile([128, 2 * TB], f32, tag="sc", name=f"sc{p}_{j}")
                    nc.tensor.matmul(
                        sc[:, 0:TB], kt[p][0:64, ts(j, 128)], qt[p][0:64, :]
                    )
                    nc.tensor.matmul(
                        sc[:, TB : 2 * TB], kt[p][64:128, ts(j, 128)], qt[p][64:128, :]
                    )
                    e = ep.tile([128, 2 * TB], bf16, tag="e", name=f"e{p}_{j}")
                    if j in fexp_js:
                        # Schraudolph fast-exp on DVE: bf16 bit pattern via
                        # int16 round(x*C1 + C2); error ~3% per weight,
                        # washes out over the 2048-key softmax average
                        nc.vector.tensor_scalar(
                            out=e[:].bitcast(i16), in0=sc[:],
                            scalar1=FEXP_C1, scalar2=FEXP_C2, op0=Mult, op1=Add,
                        )
                    else:
                        nc.scalar.activation(out=e[:], in_=sc[:], func=Exp, scale=0.125)
                    # V projection emitted after scores/exp so a late wv/xt
                    # DMA can't block the PE stream ahead of the scores
                    if p == 0:
                        proj_v(j)
                    # attnV deferred one iteration: the tensor queue is
                    # strict FIFO, so attnV(j) right here would block the
                    # already-queued sc(j+1) behind exp(j). One iteration
                    # of skew keeps PE from ever waiting on the exp stream.
                    if pend is not None:
                        attn_v(*pend)
                    pend = (j, e)
                # pair 1's critical projections lead the boundary: their PE
                # chains fill the exp15 wait and their DVE bias adds (which
                # gate pair 1's first scores) queue ahead of the evacuation
                # traffic
                if p == 0:
                    proj_q(1)
                    proj_k(1, 0)
                attn_v(*pend)
                # denominator rows first (they feed the next pair's recip),
                # then evacuate both accumulators into one [128, TB] SBUF
                # tile (head 2p rows 0-63, head 2p+1 rows 64-127)
                nc.vector.tensor_copy(out=dn_cur[0:1, :], in_=a0[64:65, :])
                nc.vector.tensor_copy(out=dn_cur[32:33, :], in_=a1[64:65, :])
                prev_dn = dn_cur
                if p < NCH - 1:
                    asb = npool.tile([128, TB], f32, tag="asb", bufs=2, name=f"asb_{p}")
                    nc.vector.tensor_copy(out=asb[0:64, :], in_=a0[0:64, :])
                    nc.scalar.copy(out=asb[64:128, :], in_=a1[0:64, :])
                    prev = asb
                else:
                    # pair 3 skips the SBUF evacuation entirely: the tail
                    # multiplies read the PSUM accumulators directly, so
                    # the critical chain is just dn -> recip -> broadcast
                    prev = (a0, a1)

            # ---- tail: all four fc tiles pre-accumulate bias + the first
            # three chunks (PE filler while the last pair's reciprocals run
            # on ScalarE); only the final chunk waits on normalize(3) ----
            def fc_prefill(t, fp):
                nc.tensor.matmul(
                    fp[:], ones_t[0:1, 0:128], bfct[:], start=True, stop=False
                )
                for c in range(NCH - 1):
                    nc.tensor.matmul(
                        fp[:], att[c][:, ts(t, 128)], wf[c][:],
                        start=False, stop=False,
                    )

            # fp2/fp3 share ONE scores-pool slot (each only needs half its
            # 2-bank width), leaving the other slot free for rb3 below
            fp23 = scp.tile([128, 2 * TB], f32, tag="sc", name="fp23")
            for t in (2, 3):
                fp = fp23[:, ts(t - 2, C)]
                fc_prefill(t, fp)
                fps.append(fp)
            # last pair's normalization, pipelined per fc token chunk: all
            # four [33, 128] reciprocal chunks run back-to-back on ScalarE
            # first (nothing else sits in that queue), then each chunk's
            # broadcast -> multiply -> fc -> evac -> DMA chain overlaps
            # with the next chunk's. The ot evacuations go to ScalarE
            # (free after the recips) so they never block the DVE
            # multiplies. rb3 lives in the attnV accumulator pool (free
            # after the acopies); each chunk's broadcast writes its own
            # column window of the one PSUM bank.
            rcps = []
            for t in range(NTT):
                lnt_t = npool.tile([33, 128], f32, tag="lnt", bufs=2, name=f"lnt3_{t}")
                nc.scalar.activation(out=lnt_t[:], in_=prev_dn[:, ts(t, 128)], func=Ln)
                rcp_t = npool.tile([33, 128], f16, tag="rcpt", bufs=4, name=f"rcp3_{t}")
                nc.scalar.activation(out=rcp_t[:], in_=lnt_t[:], func=Exp, scale=-1.0)
                rcps.append(rcp_t)
            a0f, a1f = prev
            rb3 = scp.tile([128, TB], f32, tag="sc", name="rb3")
            rbs = npool.tile([128, TB], f32, tag="asb", bufs=2, name="rbs")
            out_q = [nc.sync, nc.scalar, nc.gpsimd, nc.sync]
            for t in range(NTT):
                nc.tensor.matmul(rb3[:, ts(t, 128)], sel[:, :], rcps[t][:, :])
                # DVE can read only one PSUM operand, and the a0/a1
                # accumulators stay in PSUM — bounce the broadcast chunk
                # through SBUF on ScalarE (idle after the recips)
                nc.scalar.copy(out=rbs[:, ts(t, 128)], in_=rb3[:, ts(t, 128)])
                nc.vector.tensor_mul(
                    out=att[NCH - 1][0:64, ts(t, 128)],
                    in0=a0f[0:64, ts(t, 128)],
                    in1=rbs[0:64, ts(t, 128)],
                )
                nc.vector.tensor_mul(
                    out=att[NCH - 1][64:128, ts(t, 128)],
                    in0=a1f[0:64, ts(t, 128)],
                    in1=rbs[64:128, ts(t, 128)],
                )
                nc.tensor.matmul(
                    fps[t][:], att[NCH - 1][:, ts(t, 128)], wf[NCH - 1][:],
                    start=False, stop=True,
                )
            # evacuations LAST: an ot emitted mid-loop would sit in the
            # ScalarE/DVE FIFO waiting on its fc and block the next
            # chunk's broadcast-copy/multiply behind it
            for t in range(NTT):
                ot = npool.tile([128, C], bf16, tag="ot", bufs=4, name=f"ot{t}")
                if t % 2 == 0:
                    nc.vector.tensor_copy(out=ot[:], in_=fps[t][:])
                else:
                    nc.scalar.copy(out=ot[:], in_=fps[t][:])
                out_q[t].dma_start(out=out_d[ts(t, 128), :], in_=ot[:])

    _split_multi_waits(nc)
    nc.finalize()
    return nc


def get_nc():
    if "nc" not in _CACHE:
        _install_tile_drain_patch()
        _CACHE["nc"] = _build()
    return _CACHE["nc"]


def make_in_maps(x, Wq, bq, Wk, bk, Wv, bv, Wfc, bfc):
    bf = ml_dtypes.bfloat16
    x = np.asarray(x, np.float32)
    Wq, Wk, Wv, Wfc = (np.asarray(w, np.float32) for w in (Wq, Wk, Wv, Wfc))
    bq, bk, bv, bfc = (np.asarray(v, np.float32) for v in (bq, bk, bv, bfc))

    def interleave(wT):
        # [C, cols] -> [128, NCH*cols] with chunk c at columns [c*cols:...]
        cols = wT.shape[1]
        return np.ascontiguousarray(
            wT.reshape(NCH, 128, cols).transpose(1, 0, 2).reshape(128, NCH * cols)
        )

    def interleave_d(wT):
        # [C, C] -> [128, NCH*C] d-major: block (d, c) at columns
        # [(d*NCH+c)*128 : ...], so the d=0 slice [:, 0:NCH*128] ships as
        # one small early DMA
        return np.ascontiguousarray(
            wT.reshape(NCH, 128, NCH, 128).transpose(1, 2, 0, 3).reshape(128, NCH * C)
        )

    bfc_folded = (Wfc @ bv + bfc).reshape(1, C).astype(bf)
    wqT = interleave_d(np.ascontiguousarray(Wq.T).astype(bf))
    wkT = interleave_d(np.ascontiguousarray(Wk.T).astype(bf))
    wvT = interleave(np.ascontiguousarray(Wv.T).astype(bf))
    wfT = interleave(np.ascontiguousarray(Wfc.T).astype(bf))
    # q/k biases ride as 8 bf16 columns spliced in after wq's d0 block
    # (a separate [128, 8] tensor would cost a 128-row DMA of its own)
    bias_c = np.concatenate(
        [bq.reshape(NCH, 128).T, bk.reshape(NCH, 128).T], axis=1
    ).astype(bf)
    wqT_aug = np.ascontiguousarray(
        np.concatenate([wqT[:, 0 : NCH * 128], bias_c, wqT[:, NCH * 128 :]], axis=1)
    )

    in_maps = []
    for core in range(N_CORES):
        b, t = divmod(core, N_TOK // TB)
        XT = np.ascontiguousarray(x[b].reshape(N_TOK, C).T).astype(bf)
        # rotate tokens so this core's block sits at columns 0:TB — the Q
        # projection then reads xt directly (attention is j-permutation
        # invariant, so K/V token order doesn't matter as long as kt and
        # vpad agree, which they do: both derive from this xt)
        perm = np.r_[t * TB : (t + 1) * TB, 0 : t * TB, (t + 1) * TB : N_TOK]
        in_maps.append(
            {
                "xt": np.ascontiguousarray(XT[:, perm]),
                "wqT": wqT_aug,
                "wkT": wkT,
                "wvT": wvT,
                "wfT": wfT,
                "bfc": bfc_folded,
            }
        )
    return in_maps


def assemble(outs):
    """outs: list of 8 dicts with 'out' (512, 512) -> (2, 512, 64, 32)."""
    per_batch = [
        np.concatenate([outs[b * 4 + t]["out"] for t in range(4)], axis=0)
        for b in range(B)
    ]
    return np.stack(per_batch).reshape(B, C, 64, 32).astype(np.float32)


def kernel(**inputs):
    from concourse.bass_utils import run_bass_kernel_spmd

    nc = get_nc()
    in_maps = make_in_maps(**inputs)
    res = run_bass_kernel_spmd(nc, in_maps, list(range(N_CORES)))
    return assemble(res.results)

